# revision 8
# baseline (speedup 1.0000x reference)
"""AdaptiveRouter (MoE routing) Trainium2 kernel — 8 NeuronCores, data-parallel.

Reference computation (per problem):
    logits  = hidden @ router_weight.T + log(softmax(importance) + eps), / temperature
    top2    -> indices + softmax over the 2 selected logits
    probs   = softmax(logits); expert_load = probs.mean(0);
    load_variance = var(expert_load, ddof=1); entropy = -(p*log(p+eps)).sum(-1).mean()

Sharding: tokens are split 8x (2048/core). router weight / importance /
temperature replicated. Global stats are assembled on host from tiny
per-core partial sums (no collectives needed).

The device kernel receives the hidden shard pre-transposed ([H, NT]) so the
contraction dim lands on SBUF partitions (fp32 has no DMA-transpose path).
"""

import sys
import numpy as np

sys.path.insert(0, "/opt/trn_rl_repo")

from contextlib import ExitStack

import concourse.bass as bass
import concourse.bacc as bacc
import concourse.mybir as mybir
import concourse.tile as tile
import concourse.masks as masks
from concourse.bass_utils import run_bass_kernel_spmd

F32 = mybir.dt.float32
U32 = mybir.dt.uint32
AF = mybir.ActivationFunctionType
ALU = mybir.AluOpType
AX = mybir.AxisListType

# Problem geometry (hardcoded per spec nn_AdaptiveRouter_50534585205486)
N, H, E = 16384, 4096, 64
NCORES = 8
NT = N // NCORES            # tokens per core (2048)
PASSES = 2                  # token passes per core
TP = NT // PASSES           # tokens per pass (1024)
BLKS = TP // 128            # 128-token blocks per pass (8)
HC = H // 128               # contraction chunks (32)
PACK = E + 4                # packed row: 64 logits | 2 weights | 2 idx
EPS = 1e-8
TOPK = 2


def build_nc():
    nc = bacc.Bacc("TRN2", target_bir_lowering=False, debug=False)

    ht = nc.dram_tensor("ht", [H, NT], F32, kind="ExternalInput")
    wt = nc.dram_tensor("wt", [H, E], F32, kind="ExternalInput")
    imp = nc.dram_tensor("imp", [1, E], F32, kind="ExternalInput")
    temp = nc.dram_tensor("temp", [1, 1], F32, kind="ExternalInput")

    out0 = nc.dram_tensor("out0", [NT, PACK], F32, kind="ExternalOutput")
    pacc_d = nc.dram_tensor("pacc", [128, E], F32, kind="ExternalOutput")
    eacc_d = nc.dram_tensor("eacc", [128, 1], F32, kind="ExternalOutput")

    ht_v = ht.rearrange("(c q) t -> c q t", q=128)
    out_v = out0.rearrange("(s j q) c -> s q j c", j=BLKS, q=128)

    with ExitStack() as ctx:
        tc = ctx.enter_context(tile.TileContext(nc))
        cpool = ctx.enter_context(tc.tile_pool(name="const", bufs=1))
        hpool = ctx.enter_context(tc.tile_pool(name="hid", bufs=4))
        lepool = ctx.enter_context(tc.tile_pool(name="le", bufs=2))
        tmpool = ctx.enter_context(tc.tile_pool(name="tm", bufs=2))
        spool = ctx.enter_context(tc.tile_pool(name="scratch", bufs=2))
        accpool = ctx.enter_context(tc.tile_pool(name="acc", bufs=1))
        ps_acc = ctx.enter_context(
            tc.tile_pool(name="psacc", bufs=2, space=bass.MemorySpace.PSUM)
        )
        ps_t = ctx.enter_context(
            tc.tile_pool(name="pst", bufs=2, space=bass.MemorySpace.PSUM)
        )

        # ---- constants / one-time prep --------------------------------
        ident = cpool.tile([128, 128], F32)
        masks.make_identity(nc, ident[:])

        wt_sb = cpool.tile([128, HC, E], F32)
        nc.sync.dma_start(wt_sb[:], wt.rearrange("(c q) e -> q c e", q=128))

        timp = cpool.tile([1, E], F32)
        nc.sync.dma_start(timp[:], imp[:, :])
        ttemp = cpool.tile([1, 1], F32)
        nc.sync.dma_start(ttemp[:], temp[:, :])

        ones_row = cpool.tile([1, 128], F32)
        nc.vector.memset(ones_row[:], 1.0)
        eps1 = cpool.tile([1, 1], F32)
        nc.vector.memset(eps1[:], EPS)
        eps128 = cpool.tile([128, 1], F32)
        nc.vector.memset(eps128[:], EPS)
        zero128 = cpool.tile([128, 1], F32)
        nc.vector.memset(zero128[:], 0.0)

        # log(softmax(importance) + eps) on partition 0
        nm = cpool.tile([1, 1], F32)
        nc.vector.reduce_max(nm[:], timp[:], axis=AX.X, negate=True)
        te = cpool.tile([1, E], F32)
        nc.scalar.activation(te[:], timp[:], AF.Exp, bias=nm[:])
        tsum = cpool.tile([1, 1], F32)
        nc.vector.reduce_sum(tsum[:], te[:], axis=AX.X)
        trcp = cpool.tile([1, 1], F32)
        nc.vector.reciprocal(trcp[:], tsum[:])
        smx = cpool.tile([1, E], F32)
        nc.vector.tensor_scalar_mul(smx[:], te[:], trcp[:])
        lbrow = cpool.tile([1, E], F32)
        nc.scalar.activation(lbrow[:], smx[:], AF.Ln, bias=eps1[:])

        # transpose bias row -> [64, 1] per-partition scalars (PE transpose)
        lb_ps = ps_t.tile([E, 1], F32, tag="pst")
        nc.tensor.transpose(lb_ps[:], lbrow[:], ident[0:1, 0:1])
        lb64 = cpool.tile([E, 1], F32)
        nc.vector.tensor_copy(lb64[:], lb_ps[:])

        # 1/temperature broadcast to [64, 1] via tiny matmul
        inv1 = cpool.tile([1, 1], F32)
        nc.vector.reciprocal(inv1[:], ttemp[:])
        iv_ps = ps_t.tile([E, 1], F32, tag="pst")
        nc.tensor.matmul(iv_ps[:], ones_row[0:1, 0:E], inv1[:], start=True, stop=True)
        invt64 = cpool.tile([E, 1], F32)
        nc.vector.tensor_copy(invt64[:], iv_ps[:])

        # global accumulators
        pacc = accpool.tile([128, E], F32)
        nc.vector.memset(pacc[:], 0.0)
        eacc = accpool.tile([128, 1], F32)
        nc.vector.memset(eacc[:], 0.0)

        # ---- main loop ------------------------------------------------
        for ps in range(PASSES):
            acc_ps = ps_acc.tile([E, TP], F32)
            for h in range(HC):
                htile = hpool.tile([128, TP], F32)
                nc.sync.dma_start(htile[:], ht_v[h, :, ps * TP:(ps + 1) * TP])
                for half in range(TP // 512):
                    nc.tensor.matmul(
                        acc_ps[:, half * 512:(half + 1) * 512],
                        wt_sb[:, h, :],
                        htile[:, half * 512:(half + 1) * 512],
                        start=(h == 0),
                        stop=(h == HC - 1),
                    )

            # biased, temperature-scaled logits (expert-major)
            le = lepool.tile([E, TP], F32)
            nc.vector.tensor_scalar(
                le[:], acc_ps[:], scalar1=lb64[:], scalar2=invt64[:],
                op0=ALU.add, op1=ALU.mult,
            )

            # transpose to token-major packed tile [128, BLKS, PACK]
            tm = tmpool.tile([128, BLKS, PACK], F32)
            for b in range(BLKS):
                tp_ps = ps_t.tile([128, E], F32, tag="pst")
                nc.tensor.transpose(
                    tp_ps[:], le[:, b * 128:(b + 1) * 128], ident[0:E, 0:E]
                )
                nc.scalar.copy(tm[:, b, 0:E], tp_ps[:])

            lg = tm[:, :, 0:E]  # [128, BLKS, E] strided view

            # softmax over experts
            nmax = spool.tile([128, BLKS], F32)
            nc.vector.reduce_max(nmax[:], lg, axis=AX.X, negate=True)
            sh = spool.tile([128, BLKS, E], F32)
            nc.vector.tensor_tensor(
                sh[:], lg,
                nmax[:].rearrange("q (a o) -> q a o", o=1).broadcast_to((128, BLKS, E)),
                op=ALU.add,
            )
            ex = spool.tile([128, BLKS, E], F32)
            nc.scalar.activation(ex[:], sh[:], AF.Exp, bias=zero128[:])
            ssum = spool.tile([128, BLKS], F32)
            nc.vector.reduce_sum(ssum[:], ex[:], axis=AX.X)
            rs = spool.tile([128, BLKS], F32)
            nc.vector.reciprocal(rs[:], ssum[:])
            pr = spool.tile([128, BLKS, E], F32)
            nc.vector.tensor_tensor(
                pr[:], ex[:],
                rs[:].rearrange("q (a o) -> q a o", o=1).broadcast_to((128, BLKS, E)),
                op=ALU.mult,
            )

            # entropy partial: sum over experts and blocks of p*log(p+eps)
            lp = spool.tile([128, BLKS, E], F32)
            nc.scalar.activation(lp[:], pr[:], AF.Ln, bias=eps128[:])
            pl = spool.tile([128, BLKS, E], F32)
            nc.vector.tensor_mul(pl[:], pr[:], lp[:])
            entp = spool.tile([128, 1], F32)
            nc.vector.reduce_sum(entp[:], pl[:], axis=AX.XY)
            nc.vector.tensor_add(eacc[:], eacc[:], entp[:])

            # expert-load partial: sum probs over the BLKS axis (tree)
            t4 = spool.tile([128, 4, E], F32)
            nc.vector.tensor_add(t4[:], pr[:, 0:4, :], pr[:, 4:8, :])
            t2 = spool.tile([128, 2, E], F32)
            nc.vector.tensor_add(t2[:], t4[:, 0:2, :], t4[:, 2:4, :])
            t1 = spool.tile([128, 1, E], F32)
            nc.vector.tensor_add(t1[:], t2[:, 0:1, :], t2[:, 1:2, :])
            nc.vector.tensor_add(pacc[:], pacc[:], t1[:, 0, :])

            # top-2 per token
            mx = spool.tile([128, BLKS, 8], F32)
            ix = spool.tile([128, BLKS, 8], U32)
            for b in range(BLKS):
                nc.vector.max(mx[:, b, :], tm[:, b, 0:E])
                nc.vector.max_index(ix[:, b, :], mx[:, b, :], tm[:, b, 0:E])

            # expert weights: softmax over the two selected logits
            d2 = spool.tile([128, BLKS, TOPK], F32)
            nc.vector.tensor_tensor(
                d2[:], mx[:, :, 0:TOPK],
                mx[:, :, 0:1].broadcast_to((128, BLKS, TOPK)),
                op=ALU.subtract,
            )
            e2 = spool.tile([128, BLKS, TOPK], F32)
            nc.scalar.activation(e2[:], d2[:], AF.Exp, bias=zero128[:])
            s2 = spool.tile([128, BLKS], F32)
            nc.vector.reduce_sum(s2[:], e2[:], axis=AX.X)
            r2 = spool.tile([128, BLKS], F32)
            nc.vector.reciprocal(r2[:], s2[:])
            nc.vector.tensor_tensor(
                tm[:, :, E:E + TOPK], e2[:],
                r2[:].rearrange("q (a o) -> q a o", o=1).broadcast_to((128, BLKS, TOPK)),
                op=ALU.mult,
            )
            # indices (uint32 -> f32 convert; values <= 63 are exact)
            nc.vector.tensor_copy(tm[:, :, E + TOPK:E + 2 * TOPK], ix[:, :, 0:TOPK])

            nc.sync.dma_start(out_v[ps], tm[:])

        nc.sync.dma_start(pacc_d[:, :], pacc[:])
        nc.sync.dma_start(eacc_d[:, :], eacc[:])

    nc.compile()
    return nc


_NC_CACHE = None


def _get_nc():
    global _NC_CACHE
    if _NC_CACHE is None:
        _NC_CACHE = build_nc()
    return _NC_CACHE


def make_in_maps(hidden_states, router_weight, expert_importance, temperature):
    hs = np.ascontiguousarray(np.asarray(hidden_states, dtype=np.float32))
    wt = np.ascontiguousarray(np.asarray(router_weight, dtype=np.float32).T)
    imp = np.asarray(expert_importance, dtype=np.float32).reshape(1, E)
    tmp = np.asarray(temperature, dtype=np.float32).reshape(1, 1)
    in_maps = []
    for c in range(NCORES):
        shard = np.ascontiguousarray(hs[c * NT:(c + 1) * NT].T)  # [H, NT]
        in_maps.append({"ht": shard, "wt": wt, "imp": imp, "temp": tmp})
    return in_maps


def postprocess(results):
    logits = np.empty((N, E), np.float32)
    idx = np.empty((N, TOPK), np.int32)
    ew = np.empty((N, TOPK), np.float32)
    load_sum = np.zeros(E, np.float64)
    ent_sum = 0.0
    for c, r in enumerate(results):
        o = r["out0"]
        logits[c * NT:(c + 1) * NT] = o[:, 0:E]
        ew[c * NT:(c + 1) * NT] = o[:, E:E + TOPK]
        idx[c * NT:(c + 1) * NT] = np.rint(o[:, E + TOPK:E + 2 * TOPK]).astype(np.int32)
        load_sum += r["pacc"].astype(np.float64).sum(axis=0)
        ent_sum += float(r["eacc"].astype(np.float64).sum())
    expert_load = (load_sum / N).astype(np.float32)
    load_var = np.float32(np.var(load_sum / N, ddof=1))
    entropy = np.float32(-ent_sum / N)
    return (logits, idx, ew, expert_load, load_var, entropy)


def kernel(hidden_states, router_weight, expert_importance, temperature, top_k):
    assert int(top_k) == TOPK
    nc = _get_nc()
    in_maps = make_in_maps(hidden_states, router_weight, expert_importance, temperature)
    res = run_bass_kernel_spmd(nc, in_maps, core_ids=list(range(NCORES)))
    return postprocess(res.results)


# revision 14
# speedup vs baseline: 1.1978x; 1.1978x over previous
"""AdaptiveRouter (MoE routing) Trainium2 kernel — 8 NeuronCores, data-parallel.

Reference computation (per problem):
    logits  = hidden @ router_weight.T + log(softmax(importance) + eps), / temperature
    top2    -> indices + softmax over the 2 selected logits
    probs   = softmax(logits); expert_load = probs.mean(0);
    load_variance = var(expert_load, ddof=1); entropy = -(p*log(p+eps)).sum(-1).mean()

Sharding: tokens are split 8x (2048/core). router weight / importance /
temperature replicated. Global stats are assembled on host from tiny
per-core partial sums (no collectives needed).

The device kernel receives the hidden shard pre-transposed ([H, NT]) so the
contraction dim lands on SBUF partitions (fp32 has no DMA-transpose path).
"""

import sys
import numpy as np

sys.path.insert(0, "/opt/trn_rl_repo")

from contextlib import ExitStack

import concourse.bass as bass
import concourse.bacc as bacc
import concourse.mybir as mybir
import concourse.tile as tile
import concourse.masks as masks
from concourse.bass_utils import run_bass_kernel_spmd

F32 = mybir.dt.float32
U32 = mybir.dt.uint32
AF = mybir.ActivationFunctionType
ALU = mybir.AluOpType
AX = mybir.AxisListType

# Problem geometry (hardcoded per spec nn_AdaptiveRouter_50534585205486)
N, H, E = 16384, 4096, 64
NCORES = 8
NT = N // NCORES            # tokens per core (2048)
PASSES = 2                  # token passes per core
TP = NT // PASSES           # tokens per pass (1024)
BLKS = TP // 128            # 128-token blocks per pass (8)
HC = H // 128               # contraction chunks (32)
PACK = E + 4                # packed row: 64 logits | 2 weights | 2 idx
EPS = 1e-8
TOPK = 2


def build_nc():
    nc = bacc.Bacc("TRN2", target_bir_lowering=False, debug=False)

    ht = nc.dram_tensor("ht", [H, NT], F32, kind="ExternalInput")
    # wt is pre-swizzled on host to [128, HC*E] so the load is contiguous
    wt = nc.dram_tensor("wt", [128, HC * E], F32, kind="ExternalInput")
    imp = nc.dram_tensor("imp", [1, E], F32, kind="ExternalInput")
    temp = nc.dram_tensor("temp", [1, 1], F32, kind="ExternalInput")

    out0 = nc.dram_tensor("out0", [NT, PACK], F32, kind="ExternalOutput")
    pacc_d = nc.dram_tensor("pacc", [128, E], F32, kind="ExternalOutput")
    eacc_d = nc.dram_tensor("eacc", [128, 1], F32, kind="ExternalOutput")

    ht_v = ht.rearrange("(c q) t -> c q t", q=128)
    out_v = out0.rearrange("(s j q) c -> s q j c", j=BLKS, q=128)

    with ExitStack() as ctx:
        tc = ctx.enter_context(tile.TileContext(nc))
        cpool = ctx.enter_context(tc.tile_pool(name="const", bufs=1))
        hpool = ctx.enter_context(tc.tile_pool(name="hid", bufs=6))
        lepool = ctx.enter_context(tc.tile_pool(name="le", bufs=2))
        tmpool = ctx.enter_context(tc.tile_pool(name="tm", bufs=2))
        spool = ctx.enter_context(tc.tile_pool(name="scratch", bufs=2))
        accpool = ctx.enter_context(tc.tile_pool(name="acc", bufs=1))
        ps_acc = ctx.enter_context(
            tc.tile_pool(name="psacc", bufs=2, space=bass.MemorySpace.PSUM)
        )
        ps_t = ctx.enter_context(
            tc.tile_pool(name="pst", bufs=2, space=bass.MemorySpace.PSUM)
        )

        # ---- constants / one-time prep --------------------------------
        wt_sb = cpool.tile([128, HC, E], F32)
        nc.sync.dma_start(wt_sb[:], wt[:, :])

        timp = cpool.tile([1, E], F32)
        nc.sync.dma_start(timp[:], imp[:, :])
        ttemp = cpool.tile([1, 1], F32)
        nc.sync.dma_start(ttemp[:], temp[:, :])

        ident = cpool.tile([128, 128], F32)
        masks.make_identity(nc, ident[:])

        ones_row = cpool.tile([1, 128], F32)
        nc.vector.memset(ones_row[:], 1.0)
        eps1 = cpool.tile([1, 1], F32)
        nc.vector.memset(eps1[:], EPS)
        eps128 = cpool.tile([128, 1], F32)
        nc.vector.memset(eps128[:], EPS)
        zero128 = cpool.tile([128, 1], F32)
        nc.vector.memset(zero128[:], 0.0)

        # log(softmax(importance) + eps) on partition 0
        nm = cpool.tile([1, 1], F32)
        nc.vector.reduce_max(nm[:], timp[:], axis=AX.X, negate=True)
        te = cpool.tile([1, E], F32)
        nc.scalar.activation(te[:], timp[:], AF.Exp, bias=nm[:])
        tsum = cpool.tile([1, 1], F32)
        nc.vector.reduce_sum(tsum[:], te[:], axis=AX.X)
        trcp = cpool.tile([1, 1], F32)
        nc.vector.reciprocal(trcp[:], tsum[:])
        smx = cpool.tile([1, E], F32)
        nc.vector.tensor_scalar_mul(smx[:], te[:], trcp[:])
        lbrow = cpool.tile([1, E], F32)
        nc.scalar.activation(lbrow[:], smx[:], AF.Ln, bias=eps1[:])

        # transpose bias row -> [64, 1] per-partition scalars (PE transpose)
        lb_ps = ps_t.tile([E, 1], F32, tag="pst")
        nc.tensor.transpose(lb_ps[:], lbrow[:], ident[0:1, 0:1])
        lb64 = cpool.tile([E, 1], F32)
        nc.vector.tensor_copy(lb64[:], lb_ps[:])

        # 1/temperature broadcast to [64, 1] via tiny matmul
        inv1 = cpool.tile([1, 1], F32)
        nc.vector.reciprocal(inv1[:], ttemp[:])
        iv_ps = ps_t.tile([E, 1], F32, tag="pst")
        nc.tensor.matmul(iv_ps[:], ones_row[0:1, 0:E], inv1[:], start=True, stop=True)
        invt64 = cpool.tile([E, 1], F32)
        nc.vector.tensor_copy(invt64[:], iv_ps[:])

        # global accumulators
        pacc = accpool.tile([128, E], F32)
        nc.vector.memset(pacc[:], 0.0)
        eacc = accpool.tile([128, 1], F32)
        nc.vector.memset(eacc[:], 0.0)

        # ---- main loop ------------------------------------------------
        for ps in range(PASSES):
            acc_ps = ps_acc.tile([E, TP], F32)
            for h in range(HC):
                htile = hpool.tile([128, TP], F32)
                nc.sync.dma_start(htile[:], ht_v[h, :, ps * TP:(ps + 1) * TP])
                for half in range(TP // 512):
                    nc.tensor.matmul(
                        acc_ps[:, half * 512:(half + 1) * 512],
                        wt_sb[:, h, :],
                        htile[:, half * 512:(half + 1) * 512],
                        start=(h == 0),
                        stop=(h == HC - 1),
                    )

            # biased, temperature-scaled logits (expert-major)
            le = lepool.tile([E, TP], F32)
            nc.vector.tensor_scalar(
                le[:], acc_ps[:], scalar1=lb64[:], scalar2=invt64[:],
                op0=ALU.add, op1=ALU.mult,
            )

            # transpose to token-major packed tile [128, BLKS, PACK]
            tm = tmpool.tile([128, BLKS, PACK], F32)
            for b in range(BLKS):
                tp_ps = ps_t.tile([128, E], F32, tag="pst")
                nc.tensor.transpose(
                    tp_ps[:], le[:, b * 128:(b + 1) * 128], ident[0:E, 0:E]
                )
                nc.vector.tensor_copy(tm[:, b, 0:E], tp_ps[:])

            lg = tm[:, :, 0:E]  # [128, BLKS, E] strided view

            # softmax over experts
            nmax = spool.tile([128, BLKS], F32)
            nc.vector.reduce_max(nmax[:], lg, axis=AX.X, negate=True)
            sh = spool.tile([128, BLKS, E], F32)
            nc.vector.tensor_tensor(
                sh[:], lg,
                nmax[:].rearrange("q (a o) -> q a o", o=1).broadcast_to((128, BLKS, E)),
                op=ALU.add,
            )
            ex = spool.tile([128, BLKS, E], F32)
            nc.scalar.activation(ex[:], sh[:], AF.Exp, bias=zero128[:])

            # top-2 per token (independent of the probs chain; its Exp is
            # issued next to the softmax Exp to avoid an ACT table swap)
            mx = spool.tile([128, BLKS, 8], F32)
            ix = spool.tile([128, BLKS, 8], U32)
            for b in range(BLKS):
                nc.vector.max(mx[:, b, :], tm[:, b, 0:E])
                nc.vector.max_index(ix[:, b, :], mx[:, b, :], tm[:, b, 0:E])
            d2 = spool.tile([128, BLKS, TOPK], F32)
            nc.vector.tensor_tensor(
                d2[:], mx[:, :, 0:TOPK],
                mx[:, :, 0:1].broadcast_to((128, BLKS, TOPK)),
                op=ALU.subtract,
            )
            e2 = spool.tile([128, BLKS, TOPK], F32)
            nc.scalar.activation(e2[:], d2[:], AF.Exp, bias=zero128[:])

            ssum = spool.tile([128, BLKS], F32)
            nc.vector.reduce_sum(ssum[:], ex[:], axis=AX.X)
            rs = spool.tile([128, BLKS], F32)
            nc.vector.reciprocal(rs[:], ssum[:])
            pr = spool.tile([128, BLKS, E], F32)
            nc.vector.tensor_tensor(
                pr[:], ex[:],
                rs[:].rearrange("q (a o) -> q a o", o=1).broadcast_to((128, BLKS, E)),
                op=ALU.mult,
            )

            # entropy partial: sum over experts and blocks of p*log(p+eps)
            lp = spool.tile([128, BLKS, E], F32)
            nc.scalar.activation(lp[:], pr[:], AF.Ln, bias=eps128[:])
            pl = spool.tile([128, BLKS, E], F32)
            nc.vector.tensor_mul(pl[:], pr[:], lp[:])
            entp = spool.tile([128, 1], F32)
            nc.vector.reduce_sum(entp[:], pl[:], axis=AX.XY)
            nc.vector.tensor_add(eacc[:], eacc[:], entp[:])

            # expert-load partial: sum probs over the BLKS axis (tree)
            t4 = spool.tile([128, 4, E], F32)
            nc.vector.tensor_add(t4[:], pr[:, 0:4, :], pr[:, 4:8, :])
            t2 = spool.tile([128, 2, E], F32)
            nc.vector.tensor_add(t2[:], t4[:, 0:2, :], t4[:, 2:4, :])
            t1 = spool.tile([128, 1, E], F32)
            nc.vector.tensor_add(t1[:], t2[:, 0:1, :], t2[:, 1:2, :])
            nc.vector.tensor_add(pacc[:], pacc[:], t1[:, 0, :])

            s2 = spool.tile([128, BLKS], F32)
            nc.vector.reduce_sum(s2[:], e2[:], axis=AX.X)
            r2 = spool.tile([128, BLKS], F32)
            nc.vector.reciprocal(r2[:], s2[:])
            nc.vector.tensor_tensor(
                tm[:, :, E:E + TOPK], e2[:],
                r2[:].rearrange("q (a o) -> q a o", o=1).broadcast_to((128, BLKS, TOPK)),
                op=ALU.mult,
            )
            # indices (uint32 -> f32 convert; values <= 63 are exact)
            nc.vector.tensor_copy(tm[:, :, E + TOPK:E + 2 * TOPK], ix[:, :, 0:TOPK])

            nc.sync.dma_start(out_v[ps], tm[:])

        nc.sync.dma_start(pacc_d[:, :], pacc[:])
        nc.sync.dma_start(eacc_d[:, :], eacc[:])

    nc.compile()
    return nc


_NC_CACHE = None


def _get_nc():
    global _NC_CACHE
    if _NC_CACHE is None:
        _NC_CACHE = build_nc()
    return _NC_CACHE


def make_in_maps(hidden_states, router_weight, expert_importance, temperature):
    hs = np.ascontiguousarray(np.asarray(hidden_states, dtype=np.float32))
    # [E, H] -> [H, E] -> [HC, 128, E] -> [128, HC, E] -> [128, HC*E]
    wt = np.ascontiguousarray(
        np.asarray(router_weight, dtype=np.float32).T
        .reshape(HC, 128, E).transpose(1, 0, 2).reshape(128, HC * E)
    )
    imp = np.asarray(expert_importance, dtype=np.float32).reshape(1, E)
    tmp = np.asarray(temperature, dtype=np.float32).reshape(1, 1)
    in_maps = []
    for c in range(NCORES):
        shard = np.ascontiguousarray(hs[c * NT:(c + 1) * NT].T)  # [H, NT]
        in_maps.append({"ht": shard, "wt": wt, "imp": imp, "temp": tmp})
    return in_maps


def postprocess(results):
    logits = np.empty((N, E), np.float32)
    idx = np.empty((N, TOPK), np.int32)
    ew = np.empty((N, TOPK), np.float32)
    load_sum = np.zeros(E, np.float64)
    ent_sum = 0.0
    for c, r in enumerate(results):
        o = r["out0"]
        logits[c * NT:(c + 1) * NT] = o[:, 0:E]
        ew[c * NT:(c + 1) * NT] = o[:, E:E + TOPK]
        idx[c * NT:(c + 1) * NT] = np.rint(o[:, E + TOPK:E + 2 * TOPK]).astype(np.int32)
        load_sum += r["pacc"].astype(np.float64).sum(axis=0)
        ent_sum += float(r["eacc"].astype(np.float64).sum())
    expert_load = (load_sum / N).astype(np.float32)
    load_var = np.float32(np.var(load_sum / N, ddof=1))
    entropy = np.float32(-ent_sum / N)
    return (logits, idx, ew, expert_load, load_var, entropy)


def kernel(hidden_states, router_weight, expert_importance, temperature, top_k):
    assert int(top_k) == TOPK
    nc = _get_nc()
    in_maps = make_in_maps(hidden_states, router_weight, expert_importance, temperature)
    res = run_bass_kernel_spmd(nc, in_maps, core_ids=list(range(NCORES)))
    return postprocess(res.results)


# revision 18
# speedup vs baseline: 1.2009x; 1.0027x over previous
"""AdaptiveRouter (MoE routing) Trainium2 kernel — 8 NeuronCores, data-parallel.

Reference computation (per problem):
    logits  = hidden @ router_weight.T + log(softmax(importance) + eps), / temperature
    top2    -> indices + softmax over the 2 selected logits
    probs   = softmax(logits); expert_load = probs.mean(0);
    load_variance = var(expert_load, ddof=1); entropy = -(p*log(p+eps)).sum(-1).mean()

Sharding: tokens are split 8x (2048/core). router weight / importance /
temperature replicated. Global stats are assembled on host from tiny
per-core partial sums (no collectives needed).

The device kernel receives the hidden shard pre-transposed ([H, NT]) so the
contraction dim lands on SBUF partitions (fp32 has no DMA-transpose path).
"""

import sys
import numpy as np

sys.path.insert(0, "/opt/trn_rl_repo")

from contextlib import ExitStack

import concourse.bass as bass
import concourse.bacc as bacc
import concourse.mybir as mybir
import concourse.tile as tile
import concourse.masks as masks
from concourse.bass_utils import run_bass_kernel_spmd

F32 = mybir.dt.float32
U32 = mybir.dt.uint32
AF = mybir.ActivationFunctionType
ALU = mybir.AluOpType
AX = mybir.AxisListType

# Problem geometry (hardcoded per spec nn_AdaptiveRouter_50534585205486)
N, H, E = 16384, 4096, 64
NCORES = 8
NT = N // NCORES            # tokens per core (2048)
PASSES = 2                  # token passes per core
TP = NT // PASSES           # tokens per pass (1024)
BLKS = TP // 128            # 128-token blocks per pass (8)
HC = H // 128               # contraction chunks (32)
PACK = E + 4                # packed row: 64 logits | 2 weights | 2 idx
EPS = 1e-8
TOPK = 2


def build_nc():
    nc = bacc.Bacc("TRN2", target_bir_lowering=False, debug=False)

    # pass-major on host: [PASSES*H, TP]; chunk reads are fully contiguous
    ht = nc.dram_tensor("ht", [PASSES * H, TP], F32, kind="ExternalInput")
    # wt is pre-swizzled on host to [128, HC*E] so the load is contiguous
    wt = nc.dram_tensor("wt", [128, HC * E], F32, kind="ExternalInput")
    imp = nc.dram_tensor("imp", [1, E], F32, kind="ExternalInput")
    temp = nc.dram_tensor("temp", [1, 1], F32, kind="ExternalInput")

    out0 = nc.dram_tensor("out0", [NT, E], F32, kind="ExternalOutput")
    # wi rows are ordered (pass, partition, block): token = ps*TP + j*128 + p
    out1 = nc.dram_tensor("out1", [NT, 4], F32, kind="ExternalOutput")
    pacc_d = nc.dram_tensor("pacc", [128, E], F32, kind="ExternalOutput")
    eacc_d = nc.dram_tensor("eacc", [128, 1], F32, kind="ExternalOutput")

    ht_v = ht.rearrange("(s c q) t -> s c q t", s=PASSES, q=128)
    out_v = out0.rearrange("(s j q) c -> s q j c", j=BLKS, q=128)
    wi_v = out1.rearrange("(s q j) c -> s q j c", j=BLKS, q=128)

    with ExitStack() as ctx:
        tc = ctx.enter_context(tile.TileContext(nc))
        cpool = ctx.enter_context(tc.tile_pool(name="const", bufs=1))
        hpool = ctx.enter_context(tc.tile_pool(name="hid", bufs=6))
        lepool = ctx.enter_context(tc.tile_pool(name="le", bufs=2))
        tmpool = ctx.enter_context(tc.tile_pool(name="tm", bufs=2))
        spool = ctx.enter_context(tc.tile_pool(name="scratch", bufs=2))
        accpool = ctx.enter_context(tc.tile_pool(name="acc", bufs=1))
        ps_acc = ctx.enter_context(
            tc.tile_pool(name="psacc", bufs=2, space=bass.MemorySpace.PSUM)
        )
        ps_t = ctx.enter_context(
            tc.tile_pool(name="pst", bufs=2, space=bass.MemorySpace.PSUM)
        )

        # ---- constants / one-time prep --------------------------------
        wt_sb = cpool.tile([128, HC, E], F32)
        nc.scalar.dma_start(wt_sb[:], wt[:, :])

        timp = cpool.tile([1, E], F32)
        nc.sync.dma_start(timp[:], imp[:, :])
        ttemp = cpool.tile([1, 1], F32)
        nc.sync.dma_start(ttemp[:], temp[:, :])

        ident = cpool.tile([128, 128], F32)
        masks.make_identity(nc, ident[:])

        ones_row = cpool.tile([1, 128], F32)
        nc.vector.memset(ones_row[:], 1.0)
        eps1 = cpool.tile([1, 1], F32)
        nc.vector.memset(eps1[:], EPS)
        eps128 = cpool.tile([128, 1], F32)
        nc.vector.memset(eps128[:], EPS)
        zero128 = cpool.tile([128, 1], F32)
        nc.vector.memset(zero128[:], 0.0)

        # log(softmax(importance) + eps) on partition 0
        nm = cpool.tile([1, 1], F32)
        nc.vector.reduce_max(nm[:], timp[:], axis=AX.X, negate=True)
        te = cpool.tile([1, E], F32)
        nc.scalar.activation(te[:], timp[:], AF.Exp, bias=nm[:])
        tsum = cpool.tile([1, 1], F32)
        nc.vector.reduce_sum(tsum[:], te[:], axis=AX.X)
        trcp = cpool.tile([1, 1], F32)
        nc.vector.reciprocal(trcp[:], tsum[:])
        smx = cpool.tile([1, E], F32)
        nc.vector.tensor_scalar_mul(smx[:], te[:], trcp[:])
        lbrow = cpool.tile([1, E], F32)
        nc.scalar.activation(lbrow[:], smx[:], AF.Ln, bias=eps1[:])

        # transpose bias row -> [64, 1] per-partition scalars (PE transpose)
        lb_ps = ps_t.tile([E, 1], F32, tag="pst")
        nc.tensor.transpose(lb_ps[:], lbrow[:], ident[0:1, 0:1])
        lb64 = cpool.tile([E, 1], F32)
        nc.vector.tensor_copy(lb64[:], lb_ps[:])

        # 1/temperature broadcast to [64, 1] via tiny matmul
        inv1 = cpool.tile([1, 1], F32)
        nc.vector.reciprocal(inv1[:], ttemp[:])
        iv_ps = ps_t.tile([E, 1], F32, tag="pst")
        nc.tensor.matmul(iv_ps[:], ones_row[0:1, 0:E], inv1[:], start=True, stop=True)
        invt64 = cpool.tile([E, 1], F32)
        nc.vector.tensor_copy(invt64[:], iv_ps[:])

        # global accumulators
        pacc = accpool.tile([128, E], F32)
        nc.vector.memset(pacc[:], 0.0)
        eacc = accpool.tile([128, 1], F32)
        nc.vector.memset(eacc[:], 0.0)

        # ---- main loop ------------------------------------------------
        for ps in range(PASSES):
            acc_ps = ps_acc.tile([E, TP], F32)
            for h in range(HC):
                htile = hpool.tile([128, TP], F32)
                nc.sync.dma_start(htile[:], ht_v[ps, h])
                for half in range(TP // 512):
                    nc.tensor.matmul(
                        acc_ps[:, half * 512:(half + 1) * 512],
                        wt_sb[:, h, :],
                        htile[:, half * 512:(half + 1) * 512],
                        start=(h == 0),
                        stop=(h == HC - 1),
                    )

            # biased, temperature-scaled logits (expert-major)
            le = lepool.tile([E, TP], F32)
            nc.vector.tensor_scalar(
                le[:], acc_ps[:], scalar1=lb64[:], scalar2=invt64[:],
                op0=ALU.add, op1=ALU.mult,
            )

            # transpose to token-major packed tile [128, BLKS, PACK]
            tm = tmpool.tile([128, BLKS, E], F32)
            for b in range(BLKS):
                tp_ps = ps_t.tile([128, E], F32, tag="pst")
                nc.tensor.transpose(
                    tp_ps[:], le[:, b * 128:(b + 1) * 128], ident[0:E, 0:E]
                )
                nc.vector.tensor_copy(tm[:, b, 0:E], tp_ps[:])

            # logits stream out while the softmax/top-k chain runs
            nc.sync.dma_start(out_v[ps], tm[:])

            lg = tm[:, :, :]
            wi = tmpool.tile([128, BLKS, 4], F32)

            # softmax over experts
            nmax = spool.tile([128, BLKS], F32)
            nc.vector.reduce_max(nmax[:], lg, axis=AX.X, negate=True)
            sh = spool.tile([128, BLKS, E], F32)
            nc.vector.tensor_tensor(
                sh[:], lg,
                nmax[:].rearrange("q (a o) -> q a o", o=1).broadcast_to((128, BLKS, E)),
                op=ALU.add,
            )
            ex = spool.tile([128, BLKS, E], F32)
            nc.scalar.activation(ex[:], sh[:], AF.Exp, bias=zero128[:])

            # top-2 per token (independent of the probs chain; its Exp is
            # issued next to the softmax Exp to avoid an ACT table swap)
            mx = spool.tile([128, BLKS, 8], F32)
            ix = spool.tile([128, BLKS, 8], U32)
            for b in range(BLKS):
                nc.vector.max(mx[:, b, :], tm[:, b, 0:E])
                nc.vector.max_index(ix[:, b, :], mx[:, b, :], tm[:, b, 0:E])
            d2 = spool.tile([128, BLKS, TOPK], F32)
            nc.vector.tensor_tensor(
                d2[:], mx[:, :, 0:TOPK],
                mx[:, :, 0:1].broadcast_to((128, BLKS, TOPK)),
                op=ALU.subtract,
            )
            e2 = spool.tile([128, BLKS, TOPK], F32)
            nc.scalar.activation(e2[:], d2[:], AF.Exp, bias=zero128[:])

            ssum = spool.tile([128, BLKS], F32)
            nc.vector.reduce_sum(ssum[:], ex[:], axis=AX.X)
            rs = spool.tile([128, BLKS], F32)
            nc.vector.reciprocal(rs[:], ssum[:])
            pr = spool.tile([128, BLKS, E], F32)
            nc.vector.tensor_tensor(
                pr[:], ex[:],
                rs[:].rearrange("q (a o) -> q a o", o=1).broadcast_to((128, BLKS, E)),
                op=ALU.mult,
            )

            # entropy partial: sum over experts and blocks of p*log(p+eps)
            lp = spool.tile([128, BLKS, E], F32)
            nc.scalar.activation(lp[:], pr[:], AF.Ln, bias=eps128[:])
            pl = spool.tile([128, BLKS, E], F32)
            nc.vector.tensor_mul(pl[:], pr[:], lp[:])
            entp = spool.tile([128, 1], F32)
            nc.vector.reduce_sum(entp[:], pl[:], axis=AX.XY)
            nc.vector.tensor_add(eacc[:], eacc[:], entp[:])

            # expert-load partial: sum probs over the BLKS axis (tree)
            t4 = spool.tile([128, 4, E], F32)
            nc.vector.tensor_add(t4[:], pr[:, 0:4, :], pr[:, 4:8, :])
            t2 = spool.tile([128, 2, E], F32)
            nc.vector.tensor_add(t2[:], t4[:, 0:2, :], t4[:, 2:4, :])
            t1 = spool.tile([128, 1, E], F32)
            nc.vector.tensor_add(t1[:], t2[:, 0:1, :], t2[:, 1:2, :])
            nc.vector.tensor_add(pacc[:], pacc[:], t1[:, 0, :])

            s2 = spool.tile([128, BLKS], F32)
            nc.vector.reduce_sum(s2[:], e2[:], axis=AX.X)
            r2 = spool.tile([128, BLKS], F32)
            nc.vector.reciprocal(r2[:], s2[:])
            nc.vector.tensor_tensor(
                wi[:, :, 0:TOPK], e2[:],
                r2[:].rearrange("q (a o) -> q a o", o=1).broadcast_to((128, BLKS, TOPK)),
                op=ALU.mult,
            )
            # indices (uint32 -> f32 convert; values <= 63 are exact)
            nc.vector.tensor_copy(wi[:, :, TOPK:2 * TOPK], ix[:, :, 0:TOPK])

            nc.sync.dma_start(wi_v[ps], wi[:])

        nc.sync.dma_start(pacc_d[:, :], pacc[:])
        nc.sync.dma_start(eacc_d[:, :], eacc[:])

    nc.compile()
    return nc


_NC_CACHE = None


def _get_nc():
    global _NC_CACHE
    if _NC_CACHE is None:
        _NC_CACHE = build_nc()
    return _NC_CACHE


def make_in_maps(hidden_states, router_weight, expert_importance, temperature):
    hs = np.ascontiguousarray(np.asarray(hidden_states, dtype=np.float32))
    # [E, H] -> [H, E] -> [HC, 128, E] -> [128, HC, E] -> [128, HC*E]
    wt = np.ascontiguousarray(
        np.asarray(router_weight, dtype=np.float32).T
        .reshape(HC, 128, E).transpose(1, 0, 2).reshape(128, HC * E)
    )
    imp = np.asarray(expert_importance, dtype=np.float32).reshape(1, E)
    tmp = np.asarray(temperature, dtype=np.float32).reshape(1, 1)
    in_maps = []
    for c in range(NCORES):
        sh = hs[c * NT:(c + 1) * NT].T  # [H, NT]
        # pass-major stack: [PASSES*H, TP], each pass block contiguous
        shard = np.ascontiguousarray(
            np.concatenate([sh[:, p * TP:(p + 1) * TP] for p in range(PASSES)], axis=0)
        )
        in_maps.append({"ht": shard, "wt": wt, "imp": imp, "temp": tmp})
    return in_maps


def postprocess(results):
    logits = np.empty((N, E), np.float32)
    idx = np.empty((N, TOPK), np.int32)
    ew = np.empty((N, TOPK), np.float32)
    load_sum = np.zeros(E, np.float64)
    ent_sum = 0.0
    for c, r in enumerate(results):
        logits[c * NT:(c + 1) * NT] = r["out0"]
        # out1 rows are (pass, partition, block)-ordered; token = ps*TP + j*128 + p
        wi = r["out1"].reshape(PASSES, 128, BLKS, 4).transpose(0, 2, 1, 3).reshape(NT, 4)
        ew[c * NT:(c + 1) * NT] = wi[:, 0:TOPK]
        idx[c * NT:(c + 1) * NT] = np.rint(wi[:, TOPK:2 * TOPK]).astype(np.int32)
        load_sum += r["pacc"].astype(np.float64).sum(axis=0)
        ent_sum += float(r["eacc"].astype(np.float64).sum())
    expert_load = (load_sum / N).astype(np.float32)
    load_var = np.float32(np.var(load_sum / N, ddof=1))
    entropy = np.float32(-ent_sum / N)
    return (logits, idx, ew, expert_load, load_var, entropy)


def kernel(hidden_states, router_weight, expert_importance, temperature, top_k):
    assert int(top_k) == TOPK
    nc = _get_nc()
    in_maps = make_in_maps(hidden_states, router_weight, expert_importance, temperature)
    res = run_bass_kernel_spmd(nc, in_maps, core_ids=list(range(NCORES)))
    return postprocess(res.results)
